# revision 18
# baseline (speedup 1.0000x reference)
import sys
import numpy as np

sys.path.insert(0, "/opt/trn_rl_repo")

import ml_dtypes

try:
    from contextlib import ExitStack
    from concourse import bass, tile, mybir, library_config
    from concourse.bass_utils import run_bass_kernel_spmd
    from concourse.library_overlay import lower_extended_insts
    from concourse.bass_types import DynSlice
    import jax as _jax
    _jax.config.update("jax_compilation_cache_dir", "/tmp/jax_neff_cache")
    _jax.config.update("jax_persistent_cache_min_compile_time_secs", 0.0)
    _jax.config.update("jax_persistent_cache_min_entry_size_bytes", -1)
    _HAVE_BASS = True
except Exception:
    _HAVE_BASS = False

BF16 = ml_dtypes.bfloat16


def _to_bf16(x):
    """f32 -> bf16 with round-to-nearest-even via uint bit tricks."""
    u = np.ascontiguousarray(x, dtype=np.float32).view(np.uint32)
    r = ((u >> 16) & 1) + np.uint32(0x7FFF)
    return ((u + r) >> 16).astype(np.uint16).view(BF16)


def _from_bf16(x):
    u = np.asarray(x).view(np.uint16).astype(np.uint32) << 16
    return u.view(np.float32)

N = 50000
D = 256
OUT = 256
SCALING = 16.0 / 8.0
M_CORES = 8
RPC = N // M_CORES            # 6250 rows per core
TILES = (RPC + 127) // 128    # 49
RPAD = TILES * 128            # 6272
HALF = N // 2                 # 25000; gather tables are [0,HALF) and [HALF,N)

_CACHE = {}


# --------------------------------------------------------------------------
# compiler workarounds: this walrus build accepts at most ONE sync-wait
# command per instruction; spill surplus waits onto same-engine nops.
# --------------------------------------------------------------------------
def _install_patches():
    if _CACHE.get("patched"):
        return

    def split_sync_waits(nc):
        for fn in nc.m.functions:
            for bb in fn.blocks:
                insts = list(bb.instructions)
                out = []
                changed = False
                for inst in insts:
                    si = inst.sync_info
                    if si is not None and len(si.on_wait) > 1:
                        waits = list(si.on_wait)
                        ups = list(si.on_update)
                        for k, w in enumerate(waits[:-1]):
                            nop = mybir.InstNoOp(
                                name=f"{inst.name}-wspill{k}", ins=[], outs=[]
                            )
                            nop.engine = inst.engine
                            nop.sync_info = mybir.SyncInfo(on_wait=[w], on_update=[])
                            out.append(nop)
                        inst.sync_info = mybir.SyncInfo(
                            on_wait=[waits[-1]], on_update=ups
                        )
                        changed = True
                    out.append(inst)
                if changed:
                    bb.instructions = out

    orig_exit = tile.TileContext.__exit__

    def patched_exit(self, *a, **kw):
        r = orig_exit(self, *a, **kw)
        split_sync_waits(self.nc)
        return r

    if not getattr(tile.TileContext, "_wait_split_patched", False):
        tile.TileContext.__exit__ = patched_exit
        tile.TileContext._wait_split_patched = True

    import concourse.bass_utils as _bu
    if not getattr(_bu, "_birsim_patched", False):
        _orig_rc = _bu.run_command

        def _rc(cmd, **kw):
            cmd = [("--enable-birsim=false" if c == "--enable-birsim=true"
                    else c) for c in cmd]
            return _orig_rc(cmd, **kw)

        _bu.run_command = _rc
        _bu._birsim_patched = True
    _CACHE["patched"] = True


# --------------------------------------------------------------------------
# host-side edge packing
# --------------------------------------------------------------------------
def _pack_edges(row, col, val, force_min_lo=False):
    """Partition edges by (core, dst tile, col half), pad each section to a
    multiple of 128 with (idx=0, dst=0, val=0) dummies, with section block
    counts shared across cores (max over cores).

    Returns per-core dict arrays + per-(tile, half) block counts [TILES, 2].
    """
    core = row // RPC
    rl = row - core * RPC
    tile_id = rl >> 7
    dst_local = (rl & 127).astype(np.int32)
    halfsel = (col >= HALF).astype(np.int32)
    coll = (col - halfsel * HALF).astype(np.int32)

    NG = M_CORES * TILES * 2
    g = (core * TILES + tile_id) * 2 + halfsel
    order = np.argsort(g, kind="stable")
    gs = g[order]
    counts = np.bincount(gs, minlength=NG).reshape(M_CORES, TILES, 2)
    nb = (np.ceil(counts / 128.0).astype(np.int32)).max(axis=0)  # [TILES, 2]
    # uniform across tiles too (enables the For_i hardware loop)
    nb[:, 0] = max(int(nb[:, 0].max()), 1)
    nb[:, 1] = max(int(nb[:, 1].max()), 1)

    colls = coll[order]
    dsts = dst_local[order]
    vals = val[order]

    cap = nb * 128                                     # [TILES, 2]
    sec_size = np.broadcast_to(cap, (M_CORES, TILES, 2)).reshape(-1)
    sec_start = np.concatenate([[0], np.cumsum(sec_size)[:-1]])
    tot_per_core = int(cap.sum())

    grp_start = np.concatenate([[0], np.cumsum(counts.reshape(-1))[:-1]])
    pos = sec_start[gs] + (np.arange(len(gs)) - grp_start[gs])

    idx_all = np.zeros(tot_per_core * M_CORES, dtype=np.int16)
    dst_all = np.zeros(tot_per_core * M_CORES, dtype=np.float32)
    val_all = np.zeros(tot_per_core * M_CORES, dtype=np.float32)
    idx_all[pos] = colls.astype(np.int16)
    dst_all[pos] = dsts
    val_all[pos] = vals

    idx_pc = idx_all.reshape(M_CORES, tot_per_core)
    dst_pc = dst_all.reshape(M_CORES, tot_per_core)
    val_pc = val_all.reshape(M_CORES, tot_per_core)
    return idx_pc, dst_pc, val_pc, nb


def _build_layout(nb_adj, nb_dlt):
    """Per-tile section layout.

    Within tile t the gathered lo tile holds [adj-lo | dlt-lo] blocks and the
    hi tile holds [adj-hi | dlt-hi]. Returns for each tile:
      (nlo, nhi, list of (src, j, is_adj)) where src is 0 for lo / 1 for hi
    plus flat index/block offsets for slicing IDX / DSTV / VAL columns.
    """
    tiles = []
    idx_off = 0
    blk_off = 0
    for t in range(TILES):
        al, ah = int(nb_adj[t, 0]), int(nb_adj[t, 1])
        dl, dh = int(nb_dlt[t, 0]), int(nb_dlt[t, 1])
        nlo, nhi = al + dl, ah + dh
        blocks = []
        for j in range(al):
            blocks.append((0, j, True))
        for j in range(dl):
            blocks.append((0, al + j, False))
        for j in range(ah):
            blocks.append((1, j, True))
        for j in range(dh):
            blocks.append((1, ah + j, False))
        tiles.append(
            dict(nlo=nlo, nhi=nhi, blocks=blocks, idx_off=idx_off, blk_off=blk_off)
        )
        idx_off += (nlo + nhi) * 128
        blk_off += nlo + nhi
    return tiles, idx_off, blk_off


def _interleave_core(idx_a, dst_a, val_a, idx_d, dst_d, val_d, nb_adj, nb_dlt):
    """Merge one core's adj/delta packed streams into the per-tile layout:
    [adj-lo | dlt-lo | adj-hi | dlt-hi] per tile, concatenated over tiles."""
    ia = []
    idd = []
    iv = []
    a_off = 0
    d_off = 0
    for t in range(TILES):
        al, ah = int(nb_adj[t, 0]) * 128, int(nb_adj[t, 1]) * 128
        dl, dh = int(nb_dlt[t, 0]) * 128, int(nb_dlt[t, 1]) * 128
        ia.append(idx_a[a_off:a_off + al]); idd.append(dst_a[a_off:a_off + al]); iv.append(val_a[a_off:a_off + al])
        ia.append(idx_d[d_off:d_off + dl]); idd.append(dst_d[d_off:d_off + dl]); iv.append(val_d[d_off:d_off + dl])
        ia.append(idx_a[a_off + al:a_off + al + ah]); idd.append(dst_a[a_off + al:a_off + al + ah]); iv.append(val_a[a_off + al:a_off + al + ah])
        ia.append(idx_d[d_off + dl:d_off + dl + dh]); idd.append(dst_d[d_off + dl:d_off + dl + dh]); iv.append(val_d[d_off + dl:d_off + dl + dh])
        a_off += al + ah
        d_off += dl + dh
    return np.concatenate(ia), np.concatenate(idd), np.concatenate(iv)


# --------------------------------------------------------------------------
# device kernel
# --------------------------------------------------------------------------
def _build_nc(NAL, NDL, NAH, NDH):
    f32 = mybir.dt.float32
    bf16 = mybir.dt.bfloat16
    i16 = mybir.dt.int16
    i8 = mybir.dt.int8
    u8 = mybir.dt.uint8

    NLO = NAL + NDL
    NHI = NAH + NDH
    NB = NLO + NHI
    totidx = TILES * NB * 128
    totblk = TILES * NB
    GCAP = 8  # max 128-blocks per dma_gather instruction

    # packed bf16 columns: [dstv | val | W(2*OUT) | iota(128) | ident(128)]
    PKW = 2 * totblk + 2 * OUT + 256
    nc = bass.Bass(num_devices=M_CORES)
    GSH16 = RPC * 2 * D // 2
    blob = nc.declare_dram_parameter(
        "blob", [GSH16 + totidx + 128 * PKW], i16, isOutput=False)
    gsh = blob[0:GSH16].bitcast(i8).rearrange("(r c) -> r c", r=RPC)
    idxd = blob[GSH16:GSH16 + totidx].rearrange("(c p) -> p c", p=16)
    pkd = blob[GSH16 + totidx:GSH16 + totidx + 128 * PKW].bitcast(
        bf16).rearrange("(p c) -> p c", p=128)
    qo = nc.declare_dram_parameter("qo", [2, RPAD, OUT], u8, isOutput=True)
    sclo = nc.declare_dram_parameter("sclo", [128, 2 * TILES], f32, isOutput=True)

    with tile.TileContext(nc) as tc, ExitStack() as ctx:
        dram = ctx.enter_context(tc.tile_pool(name="dram", bufs=1, space="DRAM"))
        gbounce = dram.tile([RPC, 2 * D], i8, name="gbounce")
        gfull = dram.tile([N, 2 * D], i8, name="gfull")

        wpool = ctx.enter_context(tc.tile_pool(name="w", bufs=1))
        pipepool = ctx.enter_context(tc.tile_pool(name="pipe", bufs=1))
        psum = ctx.enter_context(tc.psum_pool(name="acc", bufs=2))

        nc.gpsimd.load_library(library_config.mlp)

        nidx_regs = {}
        for tot in (NLO, NHI):
            for c0 in range(0, tot, GCAP):
                n = min(GCAP, tot - c0) * 128
                if n not in nidx_regs:
                    nidx_regs[n] = nc.gpsimd.to_reg(n)

        nc.sync.dma_start(gbounce[:], gsh)
        nc.gpsimd.collective_compute(
            "AllGather", mybir.AluOpType.bypass,
            replica_groups=[list(range(M_CORES))],
            ins=[gbounce.opt()], outs=[gfull.opt()],
        )

        IDX = wpool.tile([128, totidx // 16], i16, name="IDX")
        for m in range(8):
            nc.sync.dma_start(IDX[16 * m:16 * (m + 1), :], idxd)
        PK = wpool.tile([128, PKW], bf16, name="PK")
        nc.sync.dma_start(PK[:], pkd)
        DSTV = wpool.tile([128, totblk], f32, name="DSTV")
        nc.vector.tensor_copy(DSTV[:], PK[:, 0:totblk])
        VAL = wpool.tile([128, totblk], f32, name="VAL")
        nc.vector.tensor_copy(VAL[:], PK[:, totblk:2 * totblk])
        W = PK[:, 2 * totblk:2 * totblk + 2 * OUT].rearrange(
            "p (c o) -> p c o", c=2)
        IOTAh = PK[:, 2 * totblk + 2 * OUT:2 * totblk + 2 * OUT + 128]
        IOTA = wpool.tile([128, 128], f32, name="IOTA")
        nc.vector.tensor_copy(IOTA[:], IOTAh)
        IDENT = PK[:, 2 * totblk + 2 * OUT + 128:2 * totblk + 2 * OUT + 256]
        SCL = wpool.tile([128, 2 * TILES], f32, name="SCL")

        P_adj = psum.tile([128, 2 * D], f32, name="P_adj")
        P_dlt = psum.tile([128, 2 * D], f32, name="P_dlt")
        TP = psum.tile([128, D], bf16, name="TP")
        FIX = psum.tile([128, OUT], f32, name="FIX")

        def stage_load(pipe, iv):
            GL8 = pipe.intermediate_tile([128, NLO, 2 * D], i8)
            GH8 = pipe.intermediate_tile([128, NHI, 2 * D], i8)
            o = iv * (NB * 8)
            for c0 in range(0, NLO, GCAP):
                cb = min(GCAP, NLO - c0)
                nc.gpsimd.dma_gather(
                    GL8[:, c0:c0 + cb, :], gfull[0:HALF, :],
                    IDX[:, DynSlice(o + c0 * 8, cb * 8)],
                    num_idxs=cb * 128, num_idxs_reg=nidx_regs[cb * 128],
                    elem_size=2 * D)
            for c0 in range(0, NHI, GCAP):
                cb = min(GCAP, NHI - c0)
                nc.gpsimd.dma_gather(
                    GH8[:, c0:c0 + cb, :], gfull[HALF:N, :],
                    IDX[:, DynSlice(o + (NLO + c0) * 8, cb * 8)],
                    num_idxs=cb * 128, num_idxs_reg=nidx_regs[cb * 128],
                    elem_size=2 * D)
            return GL8, GH8

        def stage_compute(pipe, iv, tiles):
            GL8, GH8 = tiles
            GL = pipe.intermediate_tile([128, NLO, 2 * D], bf16)
            nc.vector.tensor_copy(GL[:], GL8[:])
            GH = pipe.intermediate_tile([128, NHI, 2 * D], bf16)
            nc.vector.tensor_copy(GH[:], GH8[:])

            ob = iv * NB
            SEQ = pipe.intermediate_tile([128, NB, 128], f32)
            nc.vector.tensor_tensor(
                SEQ[:],
                IOTA[:, :].broadcast_to([128, 128, NB]).rearrange(
                    "p c b -> p b c"),
                DSTV[:, DynSlice(ob, NB)].broadcast_to([128, NB, 128]),
                mybir.AluOpType.is_equal)
            S_all = pipe.intermediate_tile([128, NB, 128], bf16)
            nc.vector.tensor_tensor(
                S_all[:], SEQ[:],
                VAL[:, DynSlice(ob, NB)].broadcast_to([128, NB, 128]),
                mybir.AluOpType.mult)

            # block order: [adj-lo | dlt-lo | adj-hi | dlt-hi]
            n_adj = NAL + NAH
            n_dlt = NDL + NDH
            ai = di = 0
            for k in range(NB):
                if k < NAL:
                    gsrc, is_adj = GL[:, k, :], True
                elif k < NLO:
                    gsrc, is_adj = GL[:, k, :], False
                elif k < NLO + NAH:
                    gsrc, is_adj = GH[:, k - NLO, :], True
                else:
                    gsrc, is_adj = GH[:, k - NLO, :], False
                if is_adj:
                    nc.tensor.matmul(P_adj[:], S_all[:, k, :], gsrc,
                                     start=(ai == 0), stop=(ai == n_adj - 1))
                    ai += 1
                else:
                    nc.tensor.matmul(P_dlt[:], S_all[:, k, :], gsrc,
                                     start=(di == 0), stop=(di == n_dlt - 1))
                    di += 1

            # F_input = adj@dF + dadj@P ; B = adj@P + dadj@P
            dlt_sb = pipe.intermediate_tile([128, D], f32)
            nc.scalar.copy(dlt_sb[:], P_dlt[:, D:2 * D])
            fin_bf = pipe.intermediate_tile([128, D], bf16)
            nc.vector.tensor_tensor(fin_bf[:], P_adj[:, 0:D], dlt_sb[:],
                                    mybir.AluOpType.add)
            b_f = pipe.intermediate_tile([128, D], f32)
            nc.vector.tensor_tensor(b_f[:], P_adj[:, D:2 * D], dlt_sb[:],
                                    mybir.AluOpType.add)
            bmax = pipe.intermediate_tile([128, 1], f32)
            nc.vector.tensor_reduce(bmax[:], b_f[:], mybir.AxisListType.X,
                                    mybir.AluOpType.max,
                                    apply_absolute_value=True)
            nc.vector.tensor_scalar(SCL[:, DynSlice(TILES + iv, 1)], bmax[:],
                                    1e-30, 1.0 / 127.0,
                                    mybir.AluOpType.max, mybir.AluOpType.mult)
            brec = pipe.intermediate_tile([128, 1], f32)
            nc.vector.reciprocal(brec[:], SCL[:, DynSlice(TILES + iv, 1)])
            b_q = pipe.intermediate_tile([128, D], u8)
            nc.vector.tensor_scalar(b_q[:], b_f[:], brec[:], 128.0,
                                    mybir.AluOpType.mult, mybir.AluOpType.add)
            nc.scalar.dma_start(qo[1, bass.ts(iv, 128), :], b_q[:])

            nc.tensor.transpose(TP[:, 0:128], fin_bf[:, 0:128], IDENT)
            nc.tensor.transpose(TP[:, 128:256], fin_bf[:, 128:256], IDENT)
            finT = pipe.intermediate_tile([128, D], bf16)
            nc.scalar.copy(finT[:], TP[:])
            nc.tensor.matmul(FIX[:], finT[:, 0:128], W[:, 0, :],
                             start=True, stop=False)
            nc.tensor.matmul(FIX[:], finT[:, 128:256], W[:, 1, :],
                             start=False, stop=True)
            fmax = pipe.intermediate_tile([128, 1], f32)
            nc.vector.tensor_reduce(fmax[:], FIX[:], mybir.AxisListType.X,
                                    mybir.AluOpType.max,
                                    apply_absolute_value=True)
            nc.vector.tensor_scalar(SCL[:, DynSlice(iv, 1)], fmax[:],
                                    1e-30, 1.0 / 127.0,
                                    mybir.AluOpType.max, mybir.AluOpType.mult)
            frec = pipe.intermediate_tile([128, 1], f32)
            nc.vector.reciprocal(frec[:], SCL[:, DynSlice(iv, 1)])
            fix_q = pipe.intermediate_tile([128, OUT], u8)
            nc.vector.tensor_scalar(fix_q[:], FIX[:], frec[:], 128.0,
                                    mybir.AluOpType.mult, mybir.AluOpType.add)
            nc.scalar.dma_start(qo[0, bass.ts(iv, 128), :], fix_q[:])

        tc.For_i_pipelined([stage_load, stage_compute], 0, TILES,
                           pool=pipepool, unroll=2)

        nc.sync.dma_start(sclo[:, :], SCL[:])

    lower_extended_insts(nc)
    return nc


# --------------------------------------------------------------------------
# device path
# --------------------------------------------------------------------------
def _device_path(features, delta_features, adj_row, adj_col, adj_val,
                 delta_row, delta_col, delta_val, W):
    import os, time
    dbg = bool(os.environ.get("KERNEL_DEBUG_TIMING"))
    tmark = [time.perf_counter()]

    def lap(name):
        if dbg:
            t = time.perf_counter()
            print(f"  [kernel] {name}: {t - tmark[0]:.3f}s", flush=True)
            tmark[0] = t

    if not _HAVE_BASS:
        raise RuntimeError("bass unavailable")
    _install_patches()
    lap("imports+patch")

    P = features + delta_features
    rowmax = np.maximum(np.abs(delta_features).max(axis=1),
                        np.abs(P).max(axis=1))
    scale = np.maximum(rowmax, 1e-30) * (1.0 / 127.0)
    inv = (1.0 / scale).astype(np.float32)
    G = np.empty((N, 2 * D), dtype=np.int8)
    tmp = delta_features * inv[:, None]
    np.rint(tmp, out=tmp)
    G[:, :D] = np.clip(tmp, -127, 127)
    np.multiply(P, inv[:, None], out=tmp)
    np.rint(tmp, out=tmp)
    G[:, D:] = np.clip(tmp, -127, 127)
    del tmp
    adj_val = adj_val * scale[adj_col]
    delta_val = delta_val * scale[delta_col]

    lap("G build")
    idx_a, dst_a, val_a, nb_adj = _pack_edges(adj_row, adj_col, adj_val,
                                              force_min_lo=True)
    idx_d, dst_d, val_d, nb_dlt = _pack_edges(delta_row, delta_col, delta_val,
                                              force_min_lo=True)

    lap("pack edges")
    NAL, NAH = int(nb_adj[0, 0]), int(nb_adj[0, 1])
    NDL, NDH = int(nb_dlt[0, 0]), int(nb_dlt[0, 1])

    key = (NAL, NAH, NDL, NDH)
    if _CACHE.get("key") == key:
        nc = _CACHE["nc"]
    else:
        nc = _build_nc(NAL, NDL, NAH, NDH)
        _CACHE["key"] = key
        _CACHE["nc"] = nc
        lap("build_nc")

    w3 = _to_bf16(np.ascontiguousarray(
        W.reshape(2, 128, OUT).transpose(1, 0, 2)).reshape(128, 2 * OUT))
    iota = np.tile(np.arange(128, dtype=np.float32), (128, 1)).astype(BF16)
    ident = np.eye(128, dtype=np.float32).astype(BF16)

    in_maps = []
    for m in range(M_CORES):
        ii, dd, vv = _interleave_core(idx_a[m], dst_a[m], val_a[m],
                                      idx_d[m], dst_d[m], val_d[m],
                                      nb_adj, nb_dlt)
        idx16 = np.ascontiguousarray(ii.reshape(-1, 16).T)
        dstv = _to_bf16(np.ascontiguousarray(dd.reshape(-1, 128).T))
        valv = _to_bf16(np.ascontiguousarray(vv.reshape(-1, 128).T))
        pk = np.ascontiguousarray(
            np.concatenate([dstv, valv, w3, iota, ident], axis=1))
        blob = np.concatenate([
            G[m * RPC:(m + 1) * RPC].reshape(-1).view(np.int16),
            np.ascontiguousarray(idx16.T).reshape(-1),
            pk.view(np.int16).reshape(-1)])
        in_maps.append({"blob": blob})

    lap("in_maps")
    res = run_bass_kernel_spmd(nc, in_maps, list(range(M_CORES))).results
    lap("run_spmd")
    fixed = np.empty((N, OUT), dtype=np.float32)
    B = np.empty((N, OUT), dtype=np.float32)
    for m in range(M_CORES):
        scl = np.asarray(res[m]["sclo"], dtype=np.float32)  # [128, 2*TILES]
        fs = scl[:, :TILES].T.reshape(RPAD)[:RPC]
        bs = scl[:, TILES:].T.reshape(RPAD)[:RPC]
        qo = res[m]["qo"]
        fq = qo[0, :RPC].astype(np.float32)
        fq -= 128.0
        fq *= fs[:, None]
        fixed[m * RPC:(m + 1) * RPC] = fq
        bq = qo[1, :RPC].astype(np.float32)
        bq -= 128.0
        bq *= bs[:, None]
        B[m * RPC:(m + 1) * RPC] = bq
    lap("collect")
    return fixed, B


# --------------------------------------------------------------------------
# host fallback (scipy)
# --------------------------------------------------------------------------
def _host_aggregate(features, delta_features, adj_row, adj_col, adj_val,
                    delta_row, delta_col, delta_val):
    from scipy.sparse import coo_matrix
    FD = np.concatenate([features, delta_features], axis=1)
    adj = coo_matrix((adj_val, (adj_row, adj_col)), shape=(N, N)).tocsr()
    dadj = coo_matrix((delta_val, (delta_row, delta_col)), shape=(N, N)).tocsr()
    adjP = adj @ FD
    dadjP = dadj @ FD
    F_input = adjP[:, D:] + dadjP[:, :D] + dadjP[:, D:]
    B = adjP[:, :D] + F_input
    return (np.ascontiguousarray(F_input, dtype=np.float32),
            np.ascontiguousarray(B, dtype=np.float32))


def kernel(features, delta_features, adj_row, adj_col, adj_val,
           delta_row, delta_col, delta_val, W, bias, lora_A, lora_B):
    features = np.asarray(features, dtype=np.float32)
    delta_features = np.asarray(delta_features, dtype=np.float32)
    adj_row = np.asarray(adj_row, dtype=np.int32)
    adj_col = np.asarray(adj_col, dtype=np.int32)
    adj_val = np.asarray(adj_val, dtype=np.float32)
    delta_row = np.asarray(delta_row, dtype=np.int32)
    delta_col = np.asarray(delta_col, dtype=np.int32)
    delta_val = np.asarray(delta_val, dtype=np.float32)
    Wf = np.asarray(W, dtype=np.float32)
    lora_A = np.asarray(lora_A, dtype=np.float32)
    lora_B = np.asarray(lora_B, dtype=np.float32)

    try:
        fixed, B = _device_path(features, delta_features, adj_row, adj_col,
                                adj_val, delta_row, delta_col, delta_val, Wf)
    except Exception:
        F_input, B = _host_aggregate(features, delta_features, adj_row, adj_col,
                                     adj_val, delta_row, delta_col, delta_val)
        fixed = F_input @ Wf

    new_Z = fixed + (B @ (lora_A * SCALING)) @ lora_B
    return new_Z, fixed, B
